# revision 23
# baseline (speedup 1.0000x reference)
"""Trainium2 Bass kernel for the dual-branch cross-attention module.

Computation (see the module's reference):
    q1,k1,v1 = split(x @ w_qkv1); q2,k2,v2 = split(y @ w_qkv2)   (B,H,L,D)
    a1 = softmax(1 - q1 k2^T / sqrt(D));  xo = a1 @ v1
    a2 = softmax(1 - q2 k1^T / sqrt(D));  yo = a2 @ v2
    out = (xo @ w_p1 + b_p1, yo @ w_p2 + b_p2)

Sharding: batch*heads across 8 cores. Core c handles batch b=c//2 and the
8-head slice h0=(c%2)*8. Each core computes its full LxL attention and a
partial output projection over its 512 channels; the host sums the two
partials per batch and adds the bias (softmax(1-z) == softmax(-z), so the
constant shift is dropped).

Host feeds pre-transposed bf16 x^T/y^T and bf16 weight slices, so the
device does no transposes or input casts; the scalar engine runs exp only.

Self-contained: shapes/sharding hardcoded; imports only the system bass stack.
"""

import os
import sys
from contextlib import ExitStack

import numpy as np
import ml_dtypes

for _p in ("/opt/trn_rl_repo", os.path.expanduser("~/.axon_site/_ro/trn_rl_repo")):
    if os.path.isdir(_p) and _p not in sys.path:
        sys.path.insert(0, _p)

import concourse.tile as tile
from concourse import bacc, mybir
from concourse.bass_utils import run_bass_kernel_spmd

F32 = mybir.dt.float32
BF16 = mybir.dt.bfloat16
EXP = mybir.ActivationFunctionType.Exp
Bfollowing = None

L = 1024          # sequence length
DIM = 1024        # model dim
D = 64            # head dim
SCALE = D ** -0.5
PROJ = 256        # projection out dim
NCORES = 8
PAIRS = 4         # head pairs per core (8 heads / 2)
KC = 8            # contraction chunks of 128 over DIM
MC = 8            # key-position chunks of 128 over L
LWIN = 512        # query-window (matmul free dim)
NLW = L // LWIN

W_NAMES = ("wq1", "wk1", "wv1", "wq2", "wk2", "wv2")


def _build_body(nc, tc, ins, outs, ctx):
    big = ctx.enter_context(tc.tile_pool(name="big", bufs=1))
    qkp = ctx.enter_context(tc.tile_pool(name="qkp", bufs=2))
    ep = ctx.enter_context(tc.tile_pool(name="ep", bufs=6))
    smp = ctx.enter_context(tc.tile_pool(name="smp", bufs=4))
    outp = ctx.enter_context(tc.tile_pool(name="outp", bufs=2))
    mm_ps = ctx.enter_context(tc.tile_pool(name="mm_ps", bufs=2, space="PSUM"))
    st_ps = ctx.enter_context(tc.tile_pool(name="st_ps", bufs=2, space="PSUM"))
    pv_ps = ctx.enter_context(tc.tile_pool(name="pv_ps", bufs=2, space="PSUM"))

    # ---- SBUF-resident inputs (DMAed directly, already bf16/transposed) ----
    # xT[p, h, c, l'] = x[512h + l', 128c + p] — host-prepped, contiguous DMA
    xT = big.tile([128, NLW, KC, LWIN], BF16, tag="xT")
    yT = big.tile([128, NLW, KC, LWIN], BF16, tag="yT")
    w_sb = {nm: big.tile([128, KC, 512], BF16, tag=nm, name=nm)
            for nm in W_NAMES}
    wp_sb = {nm: big.tile([128, PAIRS, PROJ], BF16, tag=nm, name=nm)
             for nm in ("wp1", "wp2")}
    # v for all pairs of one branch: [key-in-tile, l-tile, pair*(64+1+64+1)]
    va = {br: big.tile([128, MC, PAIRS * 130], BF16, tag=f"va{br}",
                       name=f"va{br}")
          for br in range(2)}
    on = {}  # (pair, branch) -> [128, L] bf16 normalized O^T

    # First compute (q1 lw0, scores mc<4) needs only the h=0 halves and
    # wq1/wk2/wv1 — order DMAs so that prefix lands first. All transfers are
    # contiguous per partition (host pre-rearranged).
    xt_r = ins["xt"].rearrange("p (h c l) -> p h c l", h=NLW, c=KC)
    yt_r = ins["yt"].rearrange("p (h c l) -> p h c l", h=NLW, c=KC)
    wr = {nm: ins[nm].rearrange("p (c n) -> p c n", c=KC) for nm in W_NAMES}
    nc.sync.dma_start(out=xT[:, 0, 0:4], in_=xt_r[:, 0, 0:4])
    nc.sync.dma_start(out=w_sb["wq1"][:, 0:4, :], in_=wr["wq1"][:, 0:4, :])
    nc.sync.dma_start(out=xT[:, 0, 4:8], in_=xt_r[:, 0, 4:8])
    nc.sync.dma_start(out=w_sb["wq1"][:, 4:8, :], in_=wr["wq1"][:, 4:8, :])
    nc.sync.dma_start(out=yT[:, 0], in_=yt_r[:, 0])
    nc.sync.dma_start(out=w_sb["wk2"], in_=wr["wk2"])
    nc.sync.dma_start(out=w_sb["wv1"], in_=wr["wv1"])
    nc.sync.dma_start(out=xT[:, 1], in_=xt_r[:, 1])
    nc.sync.dma_start(out=yT[:, 1], in_=yt_r[:, 1])
    for nm in ("wk1", "wq2", "wv2"):
        nc.sync.dma_start(out=w_sb[nm], in_=wr[nm])
    for nm in ("wp1", "wp2"):
        nc.sync.dma_start(out=wp_sb[nm], in_=ins[nm].rearrange(
            "p (c n) -> p c n", c=PAIRS))
    # ones columns for the row-sum trick (cols p*130+64 and p*130+129)
    for br in range(2):
        va_g = va[br].rearrange("p m (g k) -> p m g k", g=PAIRS)
        nc.vector.memset(va_g[:, :, :, 64:65], 1.0)
        nc.vector.memset(va_g[:, :, :, 129:130], 1.0)

    def emit_qk_win(p, nm, srcT, lw, dst=None):
        """One 512-l window of a q/k head-pair slice (this pair's 128 chans)."""
        cols = slice(p * 128, (p + 1) * 128)
        if dst is None:
            dst = qkp.tile([128, L], BF16, tag=nm, name=nm)
        wt = w_sb["w" + nm]
        lsl = slice(lw * LWIN, (lw + 1) * LWIN)
        mm = mm_ps.tile([128, 512], F32, tag="mm")
        for c in range(KC):
            nc.tensor.matmul(mm, wt[:, c, cols], srcT[:, lw, c, :],
                             start=(c == 0), stop=(c == KC - 1))
        nc.scalar.copy(out=dst[:, lsl], in_=mm)
        return dst

    def emit_qk_tensor(p, nm, srcT):
        dst = emit_qk_win(p, nm, srcT, 0)
        return emit_qk_win(p, nm, srcT, 1, dst)

    def emit_v_lt(br, lt):
        """v matmul for one l-tile (all 4 pairs), split-copied into va."""
        src, wnm = (xT, "wv1") if br == 0 else (yT, "wv2")
        wt = w_sb[wnm]
        lq, lr = divmod(lt, 4)
        mm = mm_ps.tile([128, 512], F32, tag="mm")
        for c in range(KC):
            nc.tensor.matmul(mm, src[:, lq, c, lr * 128:(lr + 1) * 128],
                             wt[:, c, :], start=(c == 0), stop=(c == KC - 1))
        mm_g = mm.rearrange("p (g k) -> p g k", g=PAIRS)
        va_g = va[br][:, lt, :].rearrange("p (g k) -> p g k", g=PAIRS)
        nc.vector.tensor_copy(out=va_g[:, :, 0:64], in_=mm_g[:, :, 0:64])
        nc.vector.tensor_copy(out=va_g[:, :, 65:129], in_=mm_g[:, :, 64:128])

    def emit_attn_window(p, br, qT, kT, lw):
        lsl = slice(lw * LWIN, (lw + 1) * LWIN)
        va_b = va[br]
        pvA = pv_ps.tile([65, 512], F32, tag="pv")
        pvB = pv_ps.tile([65, 512], F32, tag="pv")
        pA = slice(p * 130, p * 130 + 65)
        pB = slice(p * 130 + 65, p * 130 + 130)
        for mc in range(MC):
            msl = slice(mc * 128, (mc + 1) * 128)
            st = st_ps.tile([128, 1024], F32, tag="st")
            # S^T[m, l] for both heads (K=64; disjoint PE row tiles overlap)
            nc.tensor.matmul(st[:, 0:512], kT[0:64, msl], qT[0:64, lsl],
                             start=True, stop=True)
            nc.tensor.matmul(st[:, 512:1024], kT[64:128, msl], qT[64:128, lsl],
                             start=True, stop=True)
            e_t = ep.tile([128, 1024], BF16, tag="e")
            nc.scalar.activation(out=e_t, in_=st, func=EXP, scale=-SCALE)
            nc.tensor.matmul(pvA, va_b[:, mc, pA], e_t[:, 0:512],
                             start=(mc == 0), stop=(mc == MC - 1))
            nc.tensor.matmul(pvB, va_b[:, mc, pB], e_t[:, 512:1024],
                             start=(mc == 0), stop=(mc == MC - 1))
        # normalize O^T rows by the row-sum (pv row 64), off the PSUM bank.
        # The psum->sbuf copies run at high priority so the pv banks free
        # before next window's first PV matmul needs them.
        work = []
        with tc.high_priority():
            for head, pv in ((0, pvA), (1, pvB)):
                ssum = smp.tile([1, 512], F32, tag="ssum")
                nc.vector.tensor_copy(out=ssum, in_=pv[64:65, :])
                pvo = smp.tile([64, 512], F32, tag="pvo")
                nc.vector.tensor_copy(out=pvo, in_=pv[0:64, :])
                work.append((head, ssum, pvo))
        for head, ssum, pvo in work:
            sb = smp.tile([64, 512], F32, tag="sb")
            nc.gpsimd.partition_broadcast(sb, ssum)
            rb = smp.tile([64, 512], F32, tag="rb")
            nc.vector.reciprocal_approx_fast(out=rb, in_=sb)
            nc.vector.tensor_mul(out=on[(p, br)][head * 64:(head + 1) * 64, lsl],
                                 in0=pvo, in1=rb)

    def emit_proj_lt(br, lt):
        wp_nm, out_nm = (("wp1", "p1"), ("wp2", "p2"))[br]
        wt = wp_sb[wp_nm]
        out_r = outs[out_nm].rearrange("(i p) n -> p i n", p=128)
        tsl = slice(lt * 128, (lt + 1) * 128)
        mm = mm_ps.tile([128, 512], F32, tag="mm")
        for pp in range(PAIRS):
            nc.tensor.matmul(mm[:, 0:PROJ], on[(pp, br)][:, tsl], wt[:, pp, :],
                             start=(pp == 0), stop=(pp == PAIRS - 1))
        ob = outp.tile([128, PROJ], F32, tag="ob")
        nc.vector.tensor_copy(out=ob, in_=mm[:, 0:PROJ])
        nc.sync.dma_start(out=out_r[:, lt, :], in_=ob)

    for p in range(PAIRS):
        for br in range(2):
            on[(p, br)] = big.tile([128, L], BF16, tag=f"on_{p}_{br}",
                                   name=f"on_{p}_{br}")

    # ---- main schedule (emission order == scheduler priority) ----
    # Get the first attention window going on a minimal data prefix
    # (xt/yt first halves), then let later qk/v/proj fill PE bubbles.
    qk = {}
    q1 = emit_qk_win(0, "q1", xT, 0)
    k2 = emit_qk_win(0, "k2", yT, 0)
    for lt in range(4):
        emit_v_lt(0, lt)
    emit_qk_win(0, "q1", xT, 1, q1)
    emit_qk_win(0, "k2", yT, 1, k2)
    for lt in range(4, MC):
        emit_v_lt(0, lt)
    qk[("q1", 0)], qk[("k2", 0)] = q1, k2
    emit_attn_window(0, 0, q1, k2, 0)
    qk[("k1", 0)] = emit_qk_tensor(0, "k1", xT)
    emit_attn_window(0, 0, q1, k2, 1)
    qk[("q2", 0)] = emit_qk_tensor(0, "q2", yT)
    for lt in range(MC):
        emit_v_lt(1, lt)

    for p in range(PAIRS):
        q1, k2 = qk[("q1", p)], qk[("k2", p)]
        q2, k1 = qk[("q2", p)], qk[("k1", p)]
        if p > 0:
            emit_attn_window(p, 0, q1, k2, 0)
            emit_attn_window(p, 0, q1, k2, 1)
        if p + 1 < PAIRS:
            qk[("q1", p + 1)] = emit_qk_tensor(p + 1, "q1", xT)
            qk[("k2", p + 1)] = emit_qk_tensor(p + 1, "k2", yT)
        if p == PAIRS - 1:
            for lt in range(4):
                emit_proj_lt(0, lt)
        emit_attn_window(p, 1, q2, k1, 0)
        if p + 1 < PAIRS:
            qk[("k1", p + 1)] = emit_qk_tensor(p + 1, "k1", xT)
            qk[("q2", p + 1)] = emit_qk_tensor(p + 1, "q2", yT)
        elif p == PAIRS - 1:
            for lt in range(4):
                emit_proj_lt(1, lt)
        emit_attn_window(p, 1, q2, k1, 1)
        if p == PAIRS - 1:
            for lt in range(4, 8):
                emit_proj_lt(0, lt)
            for lt in range(4, 8):
                emit_proj_lt(1, lt)


def build():
    nc = bacc.Bacc("TRN2", target_bir_lowering=False, debug=False,
                   num_devices=NCORES)
    ins = {}
    for nm in ("xt", "yt"):
        ins[nm] = nc.dram_tensor(nm, [128, NLW * KC * LWIN], BF16,
                                 kind="ExternalInput").ap()
    for nm in W_NAMES:
        ins[nm] = nc.dram_tensor(nm, [128, KC * 512], BF16,
                                 kind="ExternalInput").ap()
    for nm in ("wp1", "wp2"):
        ins[nm] = nc.dram_tensor(nm, [128, PAIRS * PROJ], BF16,
                                 kind="ExternalInput").ap()
    outs = {}
    for nm in ("p1", "p2"):
        outs[nm] = nc.dram_tensor(nm, [L, PROJ], F32, kind="ExternalOutput").ap()
    with tile.TileContext(nc) as tc:
        with ExitStack() as ctx:
            _build_body(nc, tc, ins, outs, ctx)
    nc.compile()
    return nc


_NC_CACHE = None


def _get_nc():
    global _NC_CACHE
    if _NC_CACHE is None:
        _NC_CACHE = build()
    return _NC_CACHE


BF = ml_dtypes.bfloat16


def _prep_xt(xb):
    """[L, DIM] -> [128, NLW*KC*LWIN] with layout [p][h][c][l']."""
    a = xb.reshape(NLW, LWIN, KC, 128).transpose(3, 0, 2, 1)
    return np.ascontiguousarray(a).astype(BF).reshape(128, NLW * KC * LWIN)


def _prep_w(w):
    """[DIM, n] -> [128, KC*n] with layout [p][c][n]."""
    n = w.shape[1]
    a = w.reshape(KC, 128, n).transpose(1, 0, 2)
    return np.ascontiguousarray(a).astype(BF).reshape(128, KC * n)


def _prep_wp(w):
    """[512, PROJ] -> [128, PAIRS*PROJ] with layout [p][c][n]."""
    a = w.reshape(PAIRS, 128, PROJ).transpose(1, 0, 2)
    return np.ascontiguousarray(a).astype(BF).reshape(128, PAIRS * PROJ)


def make_in_maps(x, y, w_qkv1, w_qkv2, w_p1, w_p2):
    """Shard the full inputs: core c -> batch c//2, head-slice (c%2)*8.

    Host pre-transposes/tiles x/y and casts everything to bf16; weight
    slices are shared across the 4 cores of each half.
    """
    xts = [_prep_xt(x[b]) for b in range(4)]
    yts = [_prep_xt(y[b]) for b in range(4)]
    wmaps = []
    for half in range(2):
        c0 = half * 512
        m = {
            "wp1": _prep_wp(w_p1[c0:c0 + 512, :]),
            "wp2": _prep_wp(w_p2[c0:c0 + 512, :]),
        }
        for wsrc, names in ((w_qkv1, ("wq1", "wk1", "wv1")),
                            (w_qkv2, ("wq2", "wk2", "wv2"))):
            for j, nm in enumerate(names):
                base = j * DIM + c0
                m[nm] = _prep_w(np.ascontiguousarray(wsrc[:, base:base + 512]))
        wmaps.append(m)
    in_maps = []
    for c in range(NCORES):
        b, half = divmod(c, 2)
        m = {"xt": xts[b], "yt": yts[b]}
        m.update(wmaps[half])
        in_maps.append(m)
    return in_maps


def run_cores(in_maps, trace=False, trace_cores=None):
    nc = _get_nc()
    return run_bass_kernel_spmd(nc, in_maps, list(range(NCORES)),
                                trace=trace, trace_cores=trace_cores)


def kernel(x, y, w_qkv1, w_qkv2, w_p1, b_p1, w_p2, b_p2):
    x = np.asarray(x, dtype=np.float32)
    y = np.asarray(y, dtype=np.float32)
    in_maps = make_in_maps(x, y, np.asarray(w_qkv1), np.asarray(w_qkv2),
                           np.asarray(w_p1), np.asarray(w_p2))
    res = run_cores(in_maps).results
    out1 = np.stack([res[2 * b]["p1"] + res[2 * b + 1]["p1"] for b in range(4)])
    out2 = np.stack([res[2 * b]["p2"] + res[2 * b + 1]["p2"] for b in range(4)])
    out1 += np.asarray(b_p1, dtype=np.float32)
    out2 += np.asarray(b_p2, dtype=np.float32)
    return out1, out2


# revision 25
# speedup vs baseline: 1.0345x; 1.0345x over previous
"""Trainium2 Bass kernel for the dual-branch cross-attention module.

Computation (see the module's reference):
    q1,k1,v1 = split(x @ w_qkv1); q2,k2,v2 = split(y @ w_qkv2)   (B,H,L,D)
    a1 = softmax(1 - q1 k2^T / sqrt(D));  xo = a1 @ v1
    a2 = softmax(1 - q2 k1^T / sqrt(D));  yo = a2 @ v2
    out = (xo @ w_p1 + b_p1, yo @ w_p2 + b_p2)

Sharding: batch*heads across 8 cores. Core c handles batch b=c//2 and the
8-head slice h0=(c%2)*8. Each core computes its full LxL attention and a
partial output projection over its 512 channels; the host sums the two
partials per batch and adds the bias (softmax(1-z) == softmax(-z), so the
constant shift is dropped).

Host feeds pre-transposed bf16 x^T/y^T and bf16 weight slices, so the
device does no transposes or input casts; the scalar engine runs exp only.

Self-contained: shapes/sharding hardcoded; imports only the system bass stack.
"""

import os
import sys
from contextlib import ExitStack

import numpy as np
import ml_dtypes

for _p in ("/opt/trn_rl_repo", os.path.expanduser("~/.axon_site/_ro/trn_rl_repo")):
    if os.path.isdir(_p) and _p not in sys.path:
        sys.path.insert(0, _p)

import concourse.tile as tile
from concourse import bacc, mybir
from concourse.bass_utils import run_bass_kernel_spmd

F32 = mybir.dt.float32
BF16 = mybir.dt.bfloat16
EXP = mybir.ActivationFunctionType.Exp
Bfollowing = None

L = 1024          # sequence length
DIM = 1024        # model dim
D = 64            # head dim
SCALE = D ** -0.5
PROJ = 256        # projection out dim
NCORES = 8
PAIRS = 4         # head pairs per core (8 heads / 2)
KC = 8            # contraction chunks of 128 over DIM
MC = 8            # key-position chunks of 128 over L
LWIN = 512        # query-window (matmul free dim)
NLW = L // LWIN

W_NAMES = ("wq1", "wk1", "wv1", "wq2", "wk2", "wv2")


def _build_body(nc, tc, ins, outs, ctx):
    big = ctx.enter_context(tc.tile_pool(name="big", bufs=1))
    qkp = ctx.enter_context(tc.tile_pool(name="qkp", bufs=2))
    ep = ctx.enter_context(tc.tile_pool(name="ep", bufs=6))
    smp = ctx.enter_context(tc.tile_pool(name="smp", bufs=4))
    outp = ctx.enter_context(tc.tile_pool(name="outp", bufs=2))
    mm_ps = ctx.enter_context(tc.tile_pool(name="mm_ps", bufs=2, space="PSUM"))
    st_ps = ctx.enter_context(tc.tile_pool(name="st_ps", bufs=2, space="PSUM"))
    pv_ps = ctx.enter_context(tc.tile_pool(name="pv_ps", bufs=2, space="PSUM"))

    # ---- SBUF-resident inputs (DMAed directly, already bf16/transposed) ----
    # xT[p, h, c, l'] = x[512h + l', 128c + p] — host-prepped, contiguous DMA
    xT = big.tile([128, NLW, KC, LWIN], BF16, tag="xT")
    yT = big.tile([128, NLW, KC, LWIN], BF16, tag="yT")
    w_sb = {nm: big.tile([128, KC, 512], BF16, tag=nm, name=nm)
            for nm in W_NAMES}
    wp_sb = {nm: big.tile([128, PAIRS, PROJ], BF16, tag=nm, name=nm)
             for nm in ("wp1", "wp2")}
    # v for all pairs of one branch: [key-in-tile, l-tile, pair*(64+1+64+1)]
    va = {br: big.tile([128, MC, PAIRS * 130], BF16, tag=f"va{br}",
                       name=f"va{br}")
          for br in range(2)}
    on = {}  # (pair, branch) -> [128, L] bf16 normalized O^T

    # First compute (q1 lw0, scores mc<4) needs only the h=0 halves and
    # wq1/wk2/wv1 — order DMAs so that prefix lands first. All transfers are
    # contiguous per partition (host pre-rearranged).
    xt_r = ins["xt"].rearrange("p (h c l) -> p h c l", h=NLW, c=KC)
    yt_r = ins["yt"].rearrange("p (h c l) -> p h c l", h=NLW, c=KC)
    wr = {nm: ins[nm].rearrange("p (c n) -> p c n", c=KC) for nm in W_NAMES}
    nc.sync.dma_start(out=xT[:, 0, 0:4], in_=xt_r[:, 0, 0:4])
    nc.sync.dma_start(out=w_sb["wq1"][:, 0:4, :], in_=wr["wq1"][:, 0:4, :])
    nc.sync.dma_start(out=xT[:, 0, 4:8], in_=xt_r[:, 0, 4:8])
    nc.sync.dma_start(out=w_sb["wq1"][:, 4:8, :], in_=wr["wq1"][:, 4:8, :])
    nc.sync.dma_start(out=yT[:, 0], in_=yt_r[:, 0])
    nc.sync.dma_start(out=w_sb["wk2"], in_=wr["wk2"])
    nc.sync.dma_start(out=w_sb["wv1"], in_=wr["wv1"])
    nc.sync.dma_start(out=xT[:, 1], in_=xt_r[:, 1])
    nc.sync.dma_start(out=yT[:, 1], in_=yt_r[:, 1])
    for nm in ("wk1", "wq2", "wv2"):
        nc.sync.dma_start(out=w_sb[nm], in_=wr[nm])
    for nm in ("wp1", "wp2"):
        nc.sync.dma_start(out=wp_sb[nm], in_=ins[nm].rearrange(
            "p (c n) -> p c n", c=PAIRS))
    # ones columns for the row-sum trick (cols p*130+64 and p*130+129)
    for br in range(2):
        va_g = va[br].rearrange("p m (g k) -> p m g k", g=PAIRS)
        nc.vector.memset(va_g[:, :, :, 64:65], 1.0)
        nc.vector.memset(va_g[:, :, :, 129:130], 1.0)

    def emit_qk_win(p, nm, srcT, lw, dst=None):
        """One 512-l window of a q/k head-pair slice (this pair's 128 chans)."""
        cols = slice(p * 128, (p + 1) * 128)
        if dst is None:
            dst = qkp.tile([128, L], BF16, tag=nm, name=nm)
        wt = w_sb["w" + nm]
        lsl = slice(lw * LWIN, (lw + 1) * LWIN)
        mm = mm_ps.tile([128, 512], F32, tag="mm")
        for c in range(KC):
            nc.tensor.matmul(mm, wt[:, c, cols], srcT[:, lw, c, :],
                             start=(c == 0), stop=(c == KC - 1))
        nc.scalar.copy(out=dst[:, lsl], in_=mm)
        return dst

    def emit_qk_tensor(p, nm, srcT):
        dst = emit_qk_win(p, nm, srcT, 0)
        return emit_qk_win(p, nm, srcT, 1, dst)

    def emit_v_lt(br, lt):
        """v matmul for one l-tile (all 4 pairs), split-copied into va."""
        src, wnm = (xT, "wv1") if br == 0 else (yT, "wv2")
        wt = w_sb[wnm]
        lq, lr = divmod(lt, 4)
        mm = mm_ps.tile([128, 512], F32, tag="mm")
        for c in range(KC):
            nc.tensor.matmul(mm, src[:, lq, c, lr * 128:(lr + 1) * 128],
                             wt[:, c, :], start=(c == 0), stop=(c == KC - 1))
        mm_g = mm.rearrange("p (g k) -> p g k", g=PAIRS)
        va_g = va[br][:, lt, :].rearrange("p (g k) -> p g k", g=PAIRS)
        nc.vector.tensor_copy(out=va_g[:, :, 0:64], in_=mm_g[:, :, 0:64])
        nc.vector.tensor_copy(out=va_g[:, :, 65:129], in_=mm_g[:, :, 64:128])

    def emit_attn_window(p, br, qT, kT, lw):
        lsl = slice(lw * LWIN, (lw + 1) * LWIN)
        va_b = va[br]
        pvA = pv_ps.tile([65, 512], F32, tag="pv")
        pvB = pv_ps.tile([65, 512], F32, tag="pv")
        pA = slice(p * 130, p * 130 + 65)
        pB = slice(p * 130 + 65, p * 130 + 130)
        for mc in range(MC):
            msl = slice(mc * 128, (mc + 1) * 128)
            st = st_ps.tile([128, 1024], F32, tag="st")
            # S^T[m, l] for both heads (K=64; disjoint PE row tiles overlap)
            nc.tensor.matmul(st[:, 0:512], kT[0:64, msl], qT[0:64, lsl],
                             start=True, stop=True)
            nc.tensor.matmul(st[:, 512:1024], kT[64:128, msl], qT[64:128, lsl],
                             start=True, stop=True)
            e_t = ep.tile([128, 1024], BF16, tag="e")
            nc.scalar.activation(out=e_t, in_=st, func=EXP, scale=-SCALE)
            nc.tensor.matmul(pvA, va_b[:, mc, pA], e_t[:, 0:512],
                             start=(mc == 0), stop=(mc == MC - 1))
            nc.tensor.matmul(pvB, va_b[:, mc, pB], e_t[:, 512:1024],
                             start=(mc == 0), stop=(mc == MC - 1))
        # normalize O^T rows by the row-sum (pv row 64), off the PSUM bank.
        # The psum->sbuf copies run at high priority so the pv banks free
        # before next window's first PV matmul needs them.
        work = []
        with tc.high_priority():
            for head, pv in ((0, pvA), (1, pvB)):
                ssum = smp.tile([1, 512], F32, tag="ssum")
                nc.vector.tensor_copy(out=ssum, in_=pv[64:65, :])
                pvo = smp.tile([64, 512], F32, tag="pvo")
                nc.vector.tensor_copy(out=pvo, in_=pv[0:64, :])
                work.append((head, ssum, pvo))
        for head, ssum, pvo in work:
            sb = smp.tile([64, 512], F32, tag="sb")
            nc.gpsimd.partition_broadcast(sb, ssum)
            rb = smp.tile([64, 512], F32, tag="rb")
            nc.vector.reciprocal_approx_fast(out=rb, in_=sb)
            nc.vector.tensor_mul(out=on[(p, br)][head * 64:(head + 1) * 64, lsl],
                                 in0=pvo, in1=rb)

    def emit_proj_lt(br, lt):
        wp_nm, out_nm = (("wp1", "p1"), ("wp2", "p2"))[br]
        wt = wp_sb[wp_nm]
        out_r = outs[out_nm].rearrange("(i p) n -> p i n", p=128)
        tsl = slice(lt * 128, (lt + 1) * 128)
        mm = mm_ps.tile([128, 512], F32, tag="mm")
        for pp in range(PAIRS):
            nc.tensor.matmul(mm[:, 0:PROJ], on[(pp, br)][:, tsl], wt[:, pp, :],
                             start=(pp == 0), stop=(pp == PAIRS - 1))
        ob = outp.tile([128, PROJ], F32, tag="ob")
        nc.scalar.copy(out=ob, in_=mm[:, 0:PROJ])
        nc.sync.dma_start(out=out_r[:, lt, :], in_=ob)

    for p in range(PAIRS):
        for br in range(2):
            on[(p, br)] = big.tile([128, L], BF16, tag=f"on_{p}_{br}",
                                   name=f"on_{p}_{br}")

    # ---- main schedule (emission order == scheduler priority) ----
    # Get the first attention window going on a minimal data prefix
    # (xt/yt first halves), then let later qk/v/proj fill PE bubbles.
    qk = {}
    q1 = emit_qk_win(0, "q1", xT, 0)
    k2 = emit_qk_win(0, "k2", yT, 0)
    for lt in range(4):
        emit_v_lt(0, lt)
    emit_qk_win(0, "q1", xT, 1, q1)
    emit_qk_win(0, "k2", yT, 1, k2)
    for lt in range(4, MC):
        emit_v_lt(0, lt)
    qk[("q1", 0)], qk[("k2", 0)] = q1, k2
    emit_attn_window(0, 0, q1, k2, 0)
    emit_attn_window(0, 0, q1, k2, 1)
    qk[("k1", 0)] = emit_qk_tensor(0, "k1", xT)
    qk[("q2", 0)] = emit_qk_tensor(0, "q2", yT)
    for lt in range(MC):
        emit_v_lt(1, lt)

    # Fillers (next pair's q/k, proj) are emitted AFTER the windows whose
    # stalls they fill: readiness lets them run in PE bubbles, while the
    # lower emission priority of window work keeps the exp pipeline fed.
    for p in range(PAIRS):
        q2, k1 = qk[("q2", p)], qk[("k1", p)]
        if p > 0:
            q1, k2 = qk[("q1", p)], qk[("k2", p)]
            emit_attn_window(p, 0, q1, k2, 0)
            emit_attn_window(p, 0, q1, k2, 1)
        emit_attn_window(p, 1, q2, k1, 0)
        emit_attn_window(p, 1, q2, k1, 1)
        if p + 1 < PAIRS:
            qk[("q1", p + 1)] = emit_qk_tensor(p + 1, "q1", xT)
            qk[("k2", p + 1)] = emit_qk_tensor(p + 1, "k2", yT)
            qk[("k1", p + 1)] = emit_qk_tensor(p + 1, "k1", xT)
            qk[("q2", p + 1)] = emit_qk_tensor(p + 1, "q2", yT)
    for lt in range(L // 128):
        emit_proj_lt(0, lt)
    for lt in range(L // 128):
        emit_proj_lt(1, lt)


def build():
    nc = bacc.Bacc("TRN2", target_bir_lowering=False, debug=False,
                   num_devices=NCORES)
    ins = {}
    for nm in ("xt", "yt"):
        ins[nm] = nc.dram_tensor(nm, [128, NLW * KC * LWIN], BF16,
                                 kind="ExternalInput").ap()
    for nm in W_NAMES:
        ins[nm] = nc.dram_tensor(nm, [128, KC * 512], BF16,
                                 kind="ExternalInput").ap()
    for nm in ("wp1", "wp2"):
        ins[nm] = nc.dram_tensor(nm, [128, PAIRS * PROJ], BF16,
                                 kind="ExternalInput").ap()
    outs = {}
    for nm in ("p1", "p2"):
        outs[nm] = nc.dram_tensor(nm, [L, PROJ], F32, kind="ExternalOutput").ap()
    with tile.TileContext(nc) as tc:
        with ExitStack() as ctx:
            _build_body(nc, tc, ins, outs, ctx)
    nc.compile()
    return nc


_NC_CACHE = None


def _get_nc():
    global _NC_CACHE
    if _NC_CACHE is None:
        _NC_CACHE = build()
    return _NC_CACHE


BF = ml_dtypes.bfloat16


def _prep_xt(xb):
    """[L, DIM] -> [128, NLW*KC*LWIN] with layout [p][h][c][l']."""
    a = xb.reshape(NLW, LWIN, KC, 128).transpose(3, 0, 2, 1)
    return np.ascontiguousarray(a).astype(BF).reshape(128, NLW * KC * LWIN)


def _prep_w(w):
    """[DIM, n] -> [128, KC*n] with layout [p][c][n]."""
    n = w.shape[1]
    a = w.reshape(KC, 128, n).transpose(1, 0, 2)
    return np.ascontiguousarray(a).astype(BF).reshape(128, KC * n)


def _prep_wp(w):
    """[512, PROJ] -> [128, PAIRS*PROJ] with layout [p][c][n]."""
    a = w.reshape(PAIRS, 128, PROJ).transpose(1, 0, 2)
    return np.ascontiguousarray(a).astype(BF).reshape(128, PAIRS * PROJ)


def make_in_maps(x, y, w_qkv1, w_qkv2, w_p1, w_p2):
    """Shard the full inputs: core c -> batch c//2, head-slice (c%2)*8.

    Host pre-transposes/tiles x/y and casts everything to bf16; weight
    slices are shared across the 4 cores of each half.
    """
    xts = [_prep_xt(x[b]) for b in range(4)]
    yts = [_prep_xt(y[b]) for b in range(4)]
    wmaps = []
    for half in range(2):
        c0 = half * 512
        m = {
            "wp1": _prep_wp(w_p1[c0:c0 + 512, :]),
            "wp2": _prep_wp(w_p2[c0:c0 + 512, :]),
        }
        for wsrc, names in ((w_qkv1, ("wq1", "wk1", "wv1")),
                            (w_qkv2, ("wq2", "wk2", "wv2"))):
            for j, nm in enumerate(names):
                base = j * DIM + c0
                m[nm] = _prep_w(np.ascontiguousarray(wsrc[:, base:base + 512]))
        wmaps.append(m)
    in_maps = []
    for c in range(NCORES):
        b, half = divmod(c, 2)
        m = {"xt": xts[b], "yt": yts[b]}
        m.update(wmaps[half])
        in_maps.append(m)
    return in_maps


def run_cores(in_maps, trace=False, trace_cores=None):
    nc = _get_nc()
    return run_bass_kernel_spmd(nc, in_maps, list(range(NCORES)),
                                trace=trace, trace_cores=trace_cores)


def kernel(x, y, w_qkv1, w_qkv2, w_p1, b_p1, w_p2, b_p2):
    x = np.asarray(x, dtype=np.float32)
    y = np.asarray(y, dtype=np.float32)
    in_maps = make_in_maps(x, y, np.asarray(w_qkv1), np.asarray(w_qkv2),
                           np.asarray(w_p1), np.asarray(w_p2))
    res = run_cores(in_maps).results
    out1 = np.stack([res[2 * b]["p1"] + res[2 * b + 1]["p1"] for b in range(4)])
    out2 = np.stack([res[2 * b]["p2"] + res[2 * b + 1]["p2"] for b in range(4)])
    out1 += np.asarray(b_p1, dtype=np.float32)
    out2 += np.asarray(b_p2, dtype=np.float32)
    return out1, out2
